# revision 20
# baseline (speedup 1.0000x reference)
"""2-layer GAT (PyG GATConv style) distributed across 8 TRN2 NeuronCores.

Sharding: nodes partitioned into 8 contiguous destination blocks (graph
parallel); weights replicated.

Per core:
  A. One matmul per 128-node tile against the extended weight matrix
     [W1 | vs1 | vd1] produces xp1 AND both attention logits (als1/ald1)
     as extra columns. Rows [xp1|als1] (260 bf16) go to a 768B-pitch
     gather table; ald1 stays SBUF-resident.
  G. AllGather of the layer-1 table.
  C. Layer-1 edge phase over the core's own destination block, one
     128-dst group at a time. Edges are packed DENSELY (one slot per
     edge, column-major [128 x CO]) sorted by (table-half, src); source
     rows arrive via batched dma_gather (int16 indices, two <32768-row
     table halves). Per-slot dst one-hots st[p,c,d] (DVE iota-compare)
     drive both the segment-sum matmuls on the TensorEngine (PSUM
     accumulation; the softmax normalizer rides as 4 extra rhs columns)
     and - transposed via PE - the per-edge broadcast of ald. The
     epilogue normalizes, applies bias+ELU, and computes the layer-2
     node row [xp2|als2] + SBUF-resident ald2 via [W2 | vs2 | vd2].
  H. AllGather of the (256B-pitch) layer-2 table.
  F. Layer-2 edge phase, same structure (17-wide rows) + log_softmax.

Host preprocessing is index-only (self-loops, dst-block bucketing,
dense slot packing, padding, int16 index images); the only host float
math is on the WEIGHTS (column permutation and folding the attention
vectors into the extended weight matrices); all data-dependent float
math runs on device.
"""

import math
from contextlib import ExitStack

import numpy as np
import ml_dtypes

import concourse.bass as bass
import concourse.tile as tile
from concourse import bacc, mybir
from concourse.bass_utils import run_bass_kernel_spmd
from concourse.masks import make_identity

F32 = mybir.dt.float32
BF16 = mybir.dt.bfloat16
I16 = mybir.dt.int16
AF = mybir.ActivationFunctionType
OP = mybir.AluOpType

P = 128
NEG_SLOPE = 0.2


def dma_gather_raw(gp, out_ap, in_ap, idxs_ap, num_idxs, elem_size,
                   elem_step, single_packet=None):
    """BassGpSimd.dma_gather minus the payload%256 assert (pitch must still
    be a 256B multiple; verified on HW that arbitrary payload works)."""
    from concourse._compat import exact_div
    if single_packet is None:
        single_packet = num_idxs <= 1024
    assert idxs_ap.dtype == mybir.dt.int16
    assert in_ap.dtype == out_ap.dtype
    stride_bytes = elem_step * mybir.dt.size(in_ap.dtype)
    stride_bytes_256 = exact_div(stride_bytes, 256)
    assert stride_bytes_256 < 256
    _in_ap = gp.lower_ap_dma(in_ap, for_custom_bir_dma=True)
    _idxs_ap = gp.lower_ap(idxs_ap)
    _out_ap = gp.lower_ap(out_ap)
    return gp.add_instruction(
        mybir.InstDMAGatherAnt(
            name=gp.bass.get_next_instruction_name(),
            ins=[*_in_ap, _idxs_ap, gp.lower_val_access(gp.to_reg(num_idxs))],
            outs=[_out_ap],
            transpose=False,
            num_idxs=num_idxs,
            elem_size=elem_size,
            stride_bytes_256=stride_bytes_256,
            gen_mode=0,
            single_packet=single_packet,
            queue_num=0,
            sbuf_tokens_per_rank=0,
            sbuf_free_dim_per_rank=0,
            sbuf_free_dim_pad_per_rank=0,
            sbuf_byte_offset=0,
        ))


class Dims:
    def __init__(self, N, E, n_cores, H1=4, C1=64, H2=1, C2=16, F_in=256):
        self.N, self.E, self.NC = N, E, n_cores
        self.F_in = F_in
        self.H1, self.C1, self.H2, self.C2 = H1, C1, H2, C2
        self.D1 = H1 * C1
        self.D2 = H2 * C2
        self.B = N // n_cores
        self.G = math.ceil(self.B / P)
        self.HALF = min(25000, (N + 1) // 2)  # int16 table split point
        self.R1 = self.D1 + self.H1          # gathered row 1: xp1|als1
        self.T1 = 384                         # table-1 pitch (768B bf16)
        self.R2 = self.D2 + self.H2          # gathered row 2: xp2|als2
        self.T2 = 128                         # table-2 pitch (256B bf16)
        self.cA = None   # per-group half-A column counts (common over cores)
        self.cB = None
        self.NI2 = None  # per-group layer-2 ap_gather num_idxs (per band)
        self.BW2 = 784   # layer-2 slab band pitch per chunk (>= ceil(B/8))
        self.M2 = 8 * self.BW2   # layer-2 slab num_elems


def _wrap_idx16(flat):
    """index list -> [128, ceil(n/16)] int16 SBUF image (16-partition wrap,
    replicated for the 8 Q7 cores)."""
    n = len(flat)
    S = math.ceil(n / 16)
    a = np.zeros((16, S), np.int16)
    i = np.arange(n)
    a[i % 16, i // 16] = flat
    return np.tile(a, (8, 1))


def host_prep(dims: Dims, edge_index: np.ndarray):
    """Index-only preprocessing: self-loops, per-core dst blocks, 128-dst
    groups, dense column-major slot packing sorted by (half, src)."""
    N, NC, B, G = dims.N, dims.NC, dims.B, dims.G
    HALF = dims.HALF
    loops = np.arange(N, dtype=np.int64)
    src = np.concatenate([edge_index[0].astype(np.int64), loops])
    dst = np.concatenate([edge_index[1].astype(np.int64), loops])

    # per (core, group): src lists split by table half, sorted by src
    lists = []           # lists[k][g] = (srcA, srcB, dloc_A, dloc_B)
    for k in range(NC):
        lo, hi = k * B, (k + 1) * B
        m = (dst >= lo) & (dst < hi)
        s_k = src[m]
        d_k = dst[m] - lo
        per_g = []
        for g in range(G):
            gm = (d_k // P) == g
            sg, dg = s_k[gm], d_k[gm] - g * P
            ha = sg < HALF
            oa = np.argsort(sg[ha], kind="stable")
            ob = np.argsort(sg[~ha], kind="stable")
            per_g.append((sg[ha][oa], sg[~ha][ob] - HALF,
                          dg[ha][oa], dg[~ha][ob]))
        lists.append(per_g)

    cA = [max(max(1, math.ceil(len(lists[k][g][0]) / P)) for k in range(NC))
          for g in range(G)]
    cB = [max(max(1, math.ceil(len(lists[k][g][1]) / P)) for k in range(NC))
          for g in range(G)]
    dims.cA, dims.cB = cA, cB

    # layer-2 banded slot structure: edge src s -> band (s%B)%8,
    # slab element m = BW2*(s//B) + (s%B)//8
    BW2 = dims.BW2
    bands = []      # bands[k][g][b] = (m_list, dloc_list)
    for k in range(NC):
        per_g = []
        for g in range(G):
            sa, sb, da, db = lists[k][g]
            s_all = np.concatenate([sa, sb + HALF])
            d_all = np.concatenate([da, db])
            bb = (s_all % B) % 8
            mm = BW2 * (s_all // B) + (s_all % B) // 8
            per_b = []
            for b in range(8):
                sel = bb == b
                o = np.argsort(mm[sel], kind="stable")
                per_b.append((mm[sel][o], d_all[sel][o]))
            per_g.append(per_b)
        bands.append(per_g)
    NI2 = []
    for g in range(G):
        n = max(len(bands[k][g][b][0]) for k in range(NC) for b in range(8))
        NI2.append(((n + 3) // 4) * 4)
    dims.NI2 = NI2
    T2g = [math.ceil(n / P) for n in NI2]
    S2 = [2 * math.ceil(n / 32) for n in NI2]
    C2 = [8 * t for t in T2g]

    SA = [c * 8 for c in cA]           # idx image cols per group (n/16)
    SB = [c * 8 for c in cB]
    CO = [a + b for a, b in zip(cA, cB)]
    per_core = []
    for k in range(NC):
        idxA = np.zeros((P, sum(SA)), np.int16)
        idxB = np.zeros((P, sum(SB)), np.int16)
        dstl = np.full((P, sum(CO)), -1.0, np.float32)
        emask = np.full((P, sum(CO)), -150.0, np.float32)
        idx2 = np.full((P, sum(S2)), -1, np.int16)
        dstl2 = np.full((P, sum(C2)), -1.0, np.float32)
        emask2 = np.full((P, sum(C2)), -150.0, np.float32)
        oa = ob = oc = o2 = oc2 = 0
        for g in range(G):
            sa, sb, da, db = lists[k][g]
            nA, nB = cA[g] * P, cB[g] * P
            fa = np.zeros(nA, np.int64)
            fa[:len(sa)] = sa
            fb = np.zeros(nB, np.int64)
            fb[:len(sb)] = sb
            idxA[:, oa:oa + SA[g]] = _wrap_idx16(fa)
            idxB[:, ob:ob + SB[g]] = _wrap_idx16(fb)
            for off, cols, dl, ne in ((0, cA[g], da, len(sa)),
                                      (cA[g], cB[g], db, len(sb))):
                i = np.arange(ne)
                dstl[i % P, oc + off + i // P] = dl
                emask[i % P, oc + off + i // P] = 0.0
            # layer-2 banded images
            ni, t2 = NI2[g], T2g[g]
            for b in range(8):
                ml, dl2 = bands[k][g][b]
                ne = len(ml)
                fm = np.full(ni, -1, np.int64)
                fm[:ne] = ml
                i = np.arange(ni)
                idx2[16 * b + i % 16, o2 + i // 16] = fm
                j = np.arange(ne)
                dstl2[j % P, oc2 + b * t2 + j // P] = dl2
                emask2[j % P, oc2 + b * t2 + j // P] = 0.0
            oa += SA[g]
            ob += SB[g]
            oc += CO[g]
            o2 += S2[g]
            oc2 += C2[g]
        per_core.append(dict(
            idxA=idxA, idxB=idxB,
            dstl=dstl.astype(ml_dtypes.bfloat16),
            emask=emask.astype(ml_dtypes.bfloat16),
            idx2=idx2,
            dstl2=dstl2.astype(ml_dtypes.bfloat16),
            emask2=emask2.astype(ml_dtypes.bfloat16),
        ))
    return per_core


def build_program(dims: Dims):
    N, NC, B, G = dims.N, dims.NC, dims.B, dims.G
    F_in, D1, D2, H1, H2 = dims.F_in, dims.D1, dims.D2, dims.H1, dims.H2
    C1 = dims.C1
    R1, T1, R2, T2 = dims.R1, dims.T1, dims.R2, dims.T2
    cA, cB = dims.cA, dims.cB
    CO = [a + b for a, b in zip(cA, cB)]
    COmax = max(CO)
    SA = [c * 8 for c in cA]
    SB = [c * 8 for c in cB]
    KF = F_in // P
    KD = D1 // P
    HALF = dims.HALF
    E1 = D1 + 2 * H1     # stage-A matmul width (xp1|als1|ald1)
    E2 = D2 + 2 * H2     # layer-2 matmul width (xp2|als2|ald2)

    NI2 = dims.NI2
    T2g = [math.ceil(n / P) for n in NI2]
    S2 = [2 * math.ceil(n / 32) for n in NI2]
    C2 = [8 * t for t in T2g]
    C2max = max(C2)
    NImax = max(NI2)
    BW2, M2 = dims.BW2, dims.M2

    nc = bacc.Bacc("TRN2", target_bir_lowering=False, debug=False,
                   enable_asserts=False, num_devices=NC)

    xT = nc.dram_tensor("xT", [F_in, B], F32, kind="ExternalInput")
    W1e = nc.dram_tensor("W1e", [F_in, E1], F32, kind="ExternalInput")
    b1 = nc.dram_tensor("b1", [D1], F32, kind="ExternalInput")
    W2e = nc.dram_tensor("W2e", [D1, E2], F32, kind="ExternalInput")
    b2 = nc.dram_tensor("b2", [D2], F32, kind="ExternalInput")
    idxA = nc.dram_tensor("idxA", [P, sum(SA)], I16, kind="ExternalInput")
    idxB = nc.dram_tensor("idxB", [P, sum(SB)], I16, kind="ExternalInput")
    dstl = nc.dram_tensor("dstl", [P, sum(CO)], BF16, kind="ExternalInput")
    emask = nc.dram_tensor("emask", [P, sum(CO)], BF16, kind="ExternalInput")
    idx2 = nc.dram_tensor("idx2", [P, sum(S2)], I16, kind="ExternalInput")
    dstl2 = nc.dram_tensor("dstl2", [P, sum(C2)], BF16, kind="ExternalInput")
    emask2 = nc.dram_tensor("emask2", [P, sum(C2)], BF16,
                            kind="ExternalInput")
    out2 = nc.dram_tensor("out2", [B, D2], F32, kind="ExternalOutput")

    t1_loc = nc.dram_tensor("t1_loc", [B, T1], BF16)
    t1_full = nc.dram_tensor("t1_full", [N, T1], BF16, addr_space="Shared")
    TS_R = 32            # f-major staging rows (18 used: xp2|als2)
    t2S_loc = nc.dram_tensor("t2S_loc", [TS_R, M2], BF16)
    t2S_full = nc.dram_tensor("t2S_full", [NC * TS_R, M2], BF16,
                              addr_space="Shared")

    rg = [list(range(NC))]

    with tile.TileContext(nc) as tc, ExitStack() as ctx:
        const = ctx.enter_context(tc.tile_pool(name="const", bufs=1))
        ictx = ExitStack()
        cpsum = ictx.enter_context(tc.tile_pool(name="cpsum", bufs=1,
                                                space="PSUM"))

        iota_i = const.tile([P, P], mybir.dt.int32, tag="iota_i")
        nc.gpsimd.iota(iota_i[:], pattern=[[1, P]], base=0,
                       channel_multiplier=0)
        iota_bf = const.tile([P, P], BF16, tag="iota_bf")
        nc.vector.tensor_copy(iota_bf[:], iota_i[:])
        ident = const.tile([P, P], BF16, tag="ident")
        make_identity(nc, ident[:])

        w1sb = const.tile([P, KF, E1], BF16, tag="w1sb")
        for c in range(KF):
            nc.gpsimd.dma_start(out=w1sb[:, c, :], in_=W1e[c * P:(c + 1) * P, :])
        w2sb = const.tile([P, KD, E2], BF16, tag="w2sb")
        for c in range(KD):
            nc.gpsimd.dma_start(out=w2sb[:, c, :], in_=W2e[c * P:(c + 1) * P, :])

        ones_row = const.tile([1, P], F32, tag="ones_row")
        nc.vector.memset(ones_row[:], 1.0)

        def replicate(vec_ap, X, tag):
            vrow = const.tile([1, X], F32, tag=tag + "_row")
            nc.sync.dma_start(out=vrow[:], in_=vec_ap[None, :])
            pr = cpsum.tile([P, X], F32, tag="reppsum")
            nc.tensor.matmul(out=pr[:], lhsT=ones_row[:], rhs=vrow[:],
                             start=True, stop=True)
            rep = const.tile([P, X], F32, tag=tag)
            nc.vector.tensor_copy(rep[:], pr[:])
            return rep

        b1_r = replicate(b1, D1, "b1_r")
        b2_r = replicate(b2, D2, "b2_r")

        idxA_sb = const.tile([P, sum(SA)], I16, tag="idxA_sb")
        nc.sync.dma_start(out=idxA_sb[:], in_=idxA[:, :])
        idxB_sb = const.tile([P, sum(SB)], I16, tag="idxB_sb")
        nc.sync.dma_start(out=idxB_sb[:], in_=idxB[:, :])
        dstl_sb = const.tile([P, sum(CO)], BF16, tag="dstl_sb")
        nc.sync.dma_start(out=dstl_sb[:], in_=dstl[:, :])
        emask_sb = const.tile([P, sum(CO)], BF16, tag="emask_sb")
        nc.sync.dma_start(out=emask_sb[:], in_=emask[:, :])
        idx2_sb = const.tile([P, sum(S2)], I16, tag="idx2_sb")
        nc.sync.dma_start(out=idx2_sb[:], in_=idx2[:, :])
        dstl2_sb = const.tile([P, sum(C2)], BF16, tag="dstl2_sb")
        nc.sync.dma_start(out=dstl2_sb[:], in_=dstl2[:, :])
        emask2_sb = const.tile([P, sum(C2)], BF16, tag="emask2_sb")
        nc.sync.dma_start(out=emask2_sb[:], in_=emask2[:, :])
        # layer-2 f-major staging (written per-group in C, dumped for AG)
        tS = const.tile([TS_R, M2], BF16, tag="tS")

        # SBUF-resident dst-side logits (written in A/C, read in C/F)
        aldt = const.tile([P, G, H1], BF16, tag="aldt")
        nc.vector.memset(aldt[:], 0.0)
        aldt2 = const.tile([P, G, H2], BF16, tag="aldt2")
        nc.vector.memset(aldt2[:], 0.0)

        ictx.close()

        # ---- stage A: layer-1 node table for own block -----------------
        actx = ExitStack()
        pa = actx.enter_context(tc.tile_pool(name="pa", bufs=3))
        pa_ps = actx.enter_context(tc.tile_pool(name="pa_ps", bufs=2,
                                                space="PSUM"))
        for t in range(G):
            n0 = t * P
            nn = min(P, B - n0)
            xta = pa.tile([P, KF, P], BF16, tag="xta")
            for c in range(KF):
                nc.gpsimd.dma_start(out=xta[:, c, :nn],
                                    in_=xT[c * P:(c + 1) * P, n0:n0 + nn])
            ps_xp = pa_ps.tile([P, E1], F32, tag="ps_xp")
            for c in range(KF):
                nc.tensor.matmul(out=ps_xp[:nn, :], lhsT=xta[:, c, :nn],
                                 rhs=w1sb[:, c, :],
                                 start=(c == 0), stop=(c == KF - 1))
            row = pa.tile([P, R1], BF16, tag="row1")
            nc.vector.tensor_copy(row[:nn, :], ps_xp[:nn, 0:R1])
            nc.vector.tensor_copy(aldt[:nn, t, :], ps_xp[:nn, R1:R1 + H1])
            nc.sync.dma_start(out=t1_loc[n0:n0 + nn, 0:R1], in_=row[:nn, :])
        actx.close()

        # ---- AllGather layer-1 table -----------------------------------
        nc.gpsimd.collective_compute(
            "AllGather", OP.bypass, replica_groups=rg,
            ins=[t1_loc.ap()], outs=[t1_full.ap()])

        # ---- stage C: layer-1 edge phase + fused layer-2 table ---------
        cctx = ExitStack()
        pg = cctx.enter_context(tc.tile_pool(name="pg", bufs=2))
        pm = cctx.enter_context(tc.tile_pool(name="pm", bufs=2))
        pe = cctx.enter_context(tc.tile_pool(name="pe", bufs=2))
        pst = cctx.enter_context(tc.tile_pool(name="pst", bufs=2))
        pt_ps = cctx.enter_context(tc.tile_pool(name="pt_ps", bufs=1,
                                                space="PSUM"))
        pc_ps = cctx.enter_context(tc.tile_pool(name="pc_ps", bufs=1,
                                                space="PSUM"))
        px_ps = cctx.enter_context(tc.tile_pool(name="px_ps", bufs=1,
                                                space="PSUM"))
        oa = ob = oc = 0
        for g in range(G):
            w0 = g * P
            wn = min(P, B - w0)
            ca, cb, co = cA[g], cB[g], CO[g]

            gat = pg.tile([P, COmax, R1], BF16, tag="gat")
            dma_gather_raw(nc.gpsimd, gat[:, 0:ca, :], t1_full[0:HALF, 0:R1],
                           idxA_sb[:, oa:oa + SA[g]], ca * P, R1, T1,
                           single_packet=False)
            dma_gather_raw(nc.gpsimd, gat[:, ca:co, :], t1_full[HALF:N, 0:R1],
                           idxB_sb[:, ob:ob + SB[g]], cb * P, R1, T1,
                           single_packet=False)

            # per-slot dst one-hots (st) and their transposes (stT)
            st = pst.tile([P, COmax, P], BF16, tag="st")
            nc.vector.tensor_tensor(
                out=st[:, 0:co, :],
                in0=iota_bf[:, None, :].to_broadcast([P, co, P]),
                in1=dstl_sb[:, oc:oc + co][:, :, None].to_broadcast(
                    [P, co, P]),
                op=OP.is_equal)
            stT_ps = pt_ps.tile([P, COmax, P], BF16, tag="stT_ps")
            for c in range(co):
                nc.tensor.transpose(stT_ps[:, c, :], st[:, c, :], ident[:])
            stT = pst.tile([P, COmax, P], BF16, tag="stT")
            nc.vector.tensor_copy(stT[:, 0:co, :], stT_ps[:, 0:co, :])

            # ald broadcast to slots: aldE[p,c,:] = ald_g[dst(p,c),:]
            aldE_ps = pt_ps.tile([P, COmax, H1], F32, tag="aldE_ps")
            for c in range(co):
                nc.tensor.matmul(out=aldE_ps[:, c, :], lhsT=stT[:, c, :],
                                 rhs=aldt[:, g, :], start=True, stop=True)
            aldE = pe.tile([P, COmax, H1], BF16, tag="aldE")
            nc.vector.tensor_copy(aldE[:, 0:co, :], aldE_ps[:, 0:co, :])

            # ex = exp(leaky_relu(als[s] + ald[d]) + pad_mask)
            ep = pe.tile([P, COmax, H1], F32, tag="ep")
            nc.vector.tensor_tensor(out=ep[:, 0:co, :],
                                    in0=gat[:, 0:co, D1:D1 + H1],
                                    in1=aldE[:, 0:co, :], op=OP.add)
            lr = pe.tile([P, COmax, H1], F32, tag="lr")
            nc.vector.tensor_scalar_mul(lr[:, 0:co, :], ep[:, 0:co, :],
                                        NEG_SLOPE)
            nc.vector.tensor_tensor(out=lr[:, 0:co, :], in0=lr[:, 0:co, :],
                                    in1=ep[:, 0:co, :], op=OP.max)
            nc.vector.tensor_tensor(
                out=lr[:, 0:co, :], in0=lr[:, 0:co, :],
                in1=emask_sb[:, oc:oc + co][:, :, None].to_broadcast(
                    [P, co, H1]),
                op=OP.add)
            msg = pm.tile([P, COmax, R1], BF16, tag="msg")
            nc.scalar.activation(msg[:, 0:co, D1:D1 + H1], lr[:, 0:co, :],
                                 AF.Exp)
            nc.vector.tensor_tensor(
                out=msg[:, 0:co, 0:D1].rearrange("p k (c h) -> p k c h",
                                                 h=H1),
                in0=gat[:, 0:co, 0:D1].rearrange("p k (c h) -> p k c h",
                                                 h=H1),
                in1=msg[:, 0:co, D1:D1 + H1][:, :, None, :].to_broadcast(
                    [P, co, C1, H1]),
                op=OP.mult)

            ps_g = pc_ps.tile([P, R1], F32, tag="ps_g")
            for c in range(co):
                nc.tensor.matmul(out=ps_g[:], lhsT=st[:, c, :],
                                 rhs=msg[:, c, :],
                                 start=(c == 0), stop=(c == co - 1))

            # epilogue: alpha-normalize, +b1, ELU -> h1 (bf16)
            rec = pe.tile([P, H1], F32, tag="rec")
            nc.vector.reciprocal(rec[:wn], ps_g[:wn, D1:D1 + H1])
            h1f = pg.tile([P, D1], F32, tag="h1f")
            nc.vector.tensor_tensor(
                out=h1f[:wn].rearrange("p (c h) -> p c h", h=H1),
                in0=ps_g[:wn, 0:D1].rearrange("p (c h) -> p c h", h=H1),
                in1=rec[:wn][:, None, :].to_broadcast([wn, C1, H1]),
                op=OP.mult)
            nc.vector.tensor_tensor(out=h1f[:wn], in0=h1f[:wn], in1=b1_r[:wn],
                                    op=OP.add)
            mn = pe.tile([P, D1], F32, tag="mn")
            nc.vector.tensor_scalar_min(mn[:wn], h1f[:wn], 0.0)
            em = pe.tile([P, D1], F32, tag="em")
            nc.scalar.activation(em[:wn], mn[:wn], AF.Exp)
            nc.vector.tensor_tensor(out=h1f[:wn], in0=h1f[:wn], in1=mn[:wn],
                                    op=OP.subtract)
            nc.vector.tensor_scalar_add(em[:wn], em[:wn], -1.0)
            h1b = pg.tile([P, D1], BF16, tag="h1b")
            nc.vector.tensor_tensor(out=h1b[:wn], in0=h1f[:wn], in1=em[:wn],
                                    op=OP.add)

            # fused layer-2 node-table build (xp2|als2|ald2)
            ps_x2 = px_ps.tile([P, E2], F32, tag="ps_x2")
            for c in range(KD):
                pt = px_ps.tile([P, P], BF16, tag="pt")
                nc.tensor.transpose(pt[:], h1b[:, c * P:(c + 1) * P], ident[:])
                cpt = pe.tile([P, P], BF16, tag="cpt")
                nc.vector.tensor_copy(cpt[:], pt[:])
                nc.tensor.matmul(out=ps_x2[:], lhsT=cpt[:], rhs=w2sb[:, c, :],
                                 start=(c == 0), stop=(c == KD - 1))
            nc.vector.tensor_copy(aldt2[:wn, g, :], ps_x2[:wn, R2:R2 + H2])
            x2sb = pe.tile([P, E2], BF16, tag="x2sb")
            nc.vector.tensor_copy(x2sb[:wn, :], ps_x2[:wn, :])
            T2p = px_ps.tile([E2, P], BF16, tag="T2p")
            nc.tensor.transpose(T2p[:], x2sb[:, :], ident[:])
            # band-reorder into the f-major staging: node j of this group
            # goes to column BW2*(j%8) + 16*g + j//8
            nc.vector.tensor_copy(
                tS[0:R2 + H2, :].rearrange(
                    "f (b m) -> f b m", b=8)[:, :, 16 * g:16 * g + 16],
                T2p[0:R2 + H2, :].rearrange("f (m b) -> f b m", b=8))

            oa += SA[g]
            ob += SB[g]
            oc += CO[g]
        cctx.close()

        # ---- AllGather layer-2 f-major staging -------------------------
        nc.sync.dma_start(out=t2S_loc[:, :], in_=tS[:])
        nc.gpsimd.collective_compute(
            "AllGather", OP.bypass, replica_groups=rg,
            ins=[t2S_loc.ap()], outs=[t2S_full.ap()])

        # ---- stage F: layer-2 edge phase + log_softmax ------------------
        fctx = ExitStack()
        pslab = fctx.enter_context(tc.tile_pool(name="pslab", bufs=1))
        pf = fctx.enter_context(tc.tile_pool(name="pf", bufs=2))
        pfs = fctx.enter_context(tc.tile_pool(name="pfs", bufs=2))
        pf2_ps = fctx.enter_context(tc.tile_pool(name="pf2_ps", bufs=1,
                                                 space="PSUM"))
        pf_ps = fctx.enter_context(tc.tile_pool(name="pf_ps", bufs=2,
                                                space="PSUM"))

        # stitch the AllGathered staging into the banded f-major slab:
        # slab[16b+r, BW2*k + m, dp] = t2S_full[TS_R*k + r + 16*dp, BW2*b + m]
        slab2 = pslab.tile([P, M2, 2], BF16, tag="slab2")
        for b in range(8):
            for dp in range(2):
                rr = 16 if dp == 0 else (R2 + H2 - 16)
                for k in range(NC):
                    nc.sync.dma_start(
                        out=slab2[16 * b:16 * b + rr,
                                  BW2 * k:BW2 * (k + 1), dp],
                        in_=t2S_full[TS_R * k + 16 * dp:
                                     TS_R * k + 16 * dp + rr,
                                     BW2 * b:BW2 * (b + 1)])

        o2 = oc2 = 0
        for g in range(G):
            w0 = g * P
            wn = min(P, B - w0)
            ni, t2c, c2 = NI2[g], T2g[g], C2[g]

            g2 = pf.tile([P, max(T2g) * P, 2], BF16, tag="g2")
            if ni < t2c * P:
                nc.vector.memset(g2[:, ni:t2c * P, :], 0.0)
            nc.gpsimd.ap_gather(g2[:, 0:ni, :], slab2[:],
                                idx2_sb[:, o2:o2 + S2[g]], channels=P,
                                num_elems=M2, d=2, num_idxs=ni)

            # transpose banded f-major -> slot-major: one full [128,128]
            # transpose per (tile, dpos); out[j, 16b+r] = band-b value r
            psT = pf2_ps.tile([P, max(T2g), 2, P], BF16, tag="psT")
            for t in range(t2c):
                for dp in range(2):
                    nc.tensor.transpose(psT[:, t, dp, :],
                                        g2[:, t * P:(t + 1) * P, dp],
                                        ident[:])
            gat2 = pf.tile([P, C2max, R2], BF16, tag="gat2")
            # gat2[:, b*t2c+t, r] = psT[:, t, 0, 16b+r]  (features 0..15)
            nc.vector.tensor_copy(
                gat2[:, 0:c2, 0:16].rearrange("p (b t) f -> p b t f", b=8),
                psT[:, 0:t2c, 0, :].rearrange("p t (b f) -> p b t f", b=8))
            # feature 16 comes from dpos=1, r=0
            nc.vector.tensor_copy(
                gat2[:, 0:c2, 16:17].rearrange("p (b t) f -> p b t f", b=8),
                psT[:, 0:t2c, 1, :].rearrange(
                    "p t (b f) -> p b t f", b=8)[:, :, :, 0:1])

            st2 = pfs.tile([P, C2max, P], BF16, tag="st2")
            nc.vector.tensor_tensor(
                out=st2[:, 0:c2, :],
                in0=iota_bf[:, None, :].to_broadcast([P, c2, P]),
                in1=dstl2_sb[:, oc2:oc2 + c2][:, :, None].to_broadcast(
                    [P, c2, P]),
                op=OP.is_equal)
            stT2_ps = pf2_ps.tile([P, C2max, P], BF16, tag="stT2_ps")
            for c in range(c2):
                nc.tensor.transpose(stT2_ps[:, c, :], st2[:, c, :], ident[:])
            stT2 = pfs.tile([P, C2max, P], BF16, tag="stT2")
            nc.vector.tensor_copy(stT2[:, 0:c2, :], stT2_ps[:, 0:c2, :])

            ald2E_ps = pf2_ps.tile([P, C2max, H2], F32, tag="ald2E_ps")
            for c in range(c2):
                nc.tensor.matmul(out=ald2E_ps[:, c, :], lhsT=stT2[:, c, :],
                                 rhs=aldt2[:, g, :], start=True, stop=True)
            ald2E = pf.tile([P, C2max, H2], BF16, tag="ald2E")
            nc.vector.tensor_copy(ald2E[:, 0:c2, :], ald2E_ps[:, 0:c2, :])

            ep2 = pf.tile([P, C2max, H2], F32, tag="ep2")
            nc.vector.tensor_tensor(out=ep2[:, 0:c2, :],
                                    in0=gat2[:, 0:c2, D2:D2 + H2],
                                    in1=ald2E[:, 0:c2, :], op=OP.add)
            lr2 = pf.tile([P, C2max, H2], F32, tag="lr2")
            nc.vector.tensor_scalar_mul(lr2[:, 0:c2, :], ep2[:, 0:c2, :],
                                        NEG_SLOPE)
            nc.vector.tensor_tensor(out=lr2[:, 0:c2, :], in0=lr2[:, 0:c2, :],
                                    in1=ep2[:, 0:c2, :], op=OP.max)
            nc.vector.tensor_tensor(
                out=lr2[:, 0:c2, :], in0=lr2[:, 0:c2, :],
                in1=emask2_sb[:, oc2:oc2 + c2][:, :, None].to_broadcast(
                    [P, c2, H2]),
                op=OP.add)
            msg2 = pf.tile([P, C2max, R2], BF16, tag="msg2")
            nc.scalar.activation(msg2[:, 0:c2, D2:D2 + H2], lr2[:, 0:c2, :],
                                 AF.Exp)
            nc.vector.tensor_tensor(
                out=msg2[:, 0:c2, 0:D2],
                in0=gat2[:, 0:c2, 0:D2],
                in1=msg2[:, 0:c2, D2:D2 + H2].to_broadcast([P, c2, D2]),
                op=OP.mult)

            ps2 = pf_ps.tile([P, R2], F32, tag="ps2")
            for c in range(c2):
                nc.tensor.matmul(out=ps2[:], lhsT=st2[:, c, :],
                                 rhs=msg2[:, c, :],
                                 start=(c == 0), stop=(c == c2 - 1))

            rec2 = pf.tile([P, H2], F32, tag="rec2")
            nc.vector.reciprocal(rec2[:wn], ps2[:wn, D2:D2 + H2])
            x2 = pf.tile([P, D2], F32, tag="x2")
            nc.vector.tensor_scalar(out=x2[:wn], in0=ps2[:wn, 0:D2],
                                    scalar1=rec2[:wn], scalar2=None,
                                    op0=OP.mult)
            nc.vector.tensor_tensor(out=x2[:wn], in0=x2[:wn], in1=b2_r[:wn],
                                    op=OP.add)
            mx = pf.tile([P, 1], F32, tag="mx")
            nc.vector.tensor_reduce(out=mx[:wn], in_=x2[:wn],
                                    axis=mybir.AxisListType.X, op=OP.max)
            xs = pf.tile([P, D2], F32, tag="xs")
            nc.vector.tensor_scalar(out=xs[:wn], in0=x2[:wn], scalar1=mx[:wn],
                                    scalar2=None, op0=OP.subtract)
            es = pf.tile([P, D2], F32, tag="es")
            ssum = pf.tile([P, 1], F32, tag="ssum")
            nc.scalar.activation(es[:wn], xs[:wn], AF.Exp, accum_out=ssum[:wn])
            ls = pf.tile([P, 1], F32, tag="ls")
            nc.scalar.activation(ls[:wn], ssum[:wn], AF.Ln)
            ot = pf.tile([P, D2], F32, tag="ot")
            nc.vector.tensor_scalar(out=ot[:wn], in0=xs[:wn], scalar1=ls[:wn],
                                    scalar2=None, op0=OP.subtract)
            nc.sync.dma_start(out=out2[w0:w0 + wn, :], in_=ot[:wn, :])

            o2 += S2[g]
            oc2 += C2[g]
        fctx.close()

    nc.compile()
    return nc


def make_in_maps(dims: Dims, inputs: dict, per_core_meta):
    """Per-core input maps. Weight transforms (host, parameters only):
    columns reordered to the (c,h)-interleaved layout and the attention
    vectors folded in as extra columns of the extended weight matrices."""
    H1, C1, D1, D2 = dims.H1, dims.C1, dims.D1, dims.D2
    perm = np.arange(D1).reshape(H1, C1).T.reshape(-1)   # [h*C+c] -> [c*H+h]
    x = np.asarray(inputs["x"], dtype=np.float32)
    W1 = np.asarray(inputs["W1"], np.float32)
    W2 = np.asarray(inputs["W2"], np.float32)
    a1s = np.asarray(inputs["a1_src"], np.float32)       # [H1, C1]
    a1d = np.asarray(inputs["a1_dst"], np.float32)
    a2s = np.asarray(inputs["a2_src"], np.float32).reshape(-1)   # [D2]
    a2d = np.asarray(inputs["a2_dst"], np.float32).reshape(-1)
    # als1[n,h] = sum_c xp1[n, h*C1+c] * a1s[h,c]  ->  vs1 = W1 @ a1s-fold
    vs1 = np.stack([W1[:, h * C1:(h + 1) * C1] @ a1s[h] for h in range(H1)],
                   axis=1)
    vd1 = np.stack([W1[:, h * C1:(h + 1) * C1] @ a1d[h] for h in range(H1)],
                   axis=1)
    W1e = np.concatenate([W1[:, perm], vs1, vd1], axis=1)
    W2p = W2[perm, :]
    W2e = np.concatenate([W2p, (W2p @ a2s)[:, None], (W2p @ a2d)[:, None]],
                         axis=1)
    reps = {
        "W1e": np.ascontiguousarray(W1e),
        "b1": np.ascontiguousarray(
            np.asarray(inputs["b1"], np.float32).reshape(-1)[perm]),
        "W2e": np.ascontiguousarray(W2e),
        "b2": np.asarray(inputs["b2"], np.float32).reshape(-1),
    }
    in_maps = []
    B = dims.B
    for k in range(dims.NC):
        m = dict(reps)
        m["xT"] = np.ascontiguousarray(x[k * B:(k + 1) * B, :].T)
        m.update(per_core_meta[k])
        in_maps.append(m)
    return in_maps


_CACHE = {}


def _get_program(dims: Dims):
    key = (dims.N, dims.E, dims.NC, tuple(dims.cA), tuple(dims.cB))
    if key not in _CACHE:
        _CACHE[key] = build_program(dims)
    return _CACHE[key]


def kernel(x: np.ndarray, edge_index: np.ndarray, W1, a1_src, a1_dst, b1,
           W2, a2_src, a2_dst, b2) -> np.ndarray:
    x = np.asarray(x)
    edge_index = np.asarray(edge_index)
    dims = Dims(N=x.shape[0], E=edge_index.shape[1], n_cores=8)
    per_core = host_prep(dims, edge_index)
    nc = _get_program(dims)
    in_maps = make_in_maps(
        dims,
        dict(x=x, edge_index=edge_index, W1=W1, a1_src=a1_src, a1_dst=a1_dst,
             b1=b1, W2=W2, a2_src=a2_src, a2_dst=a2_dst, b2=b2),
        per_core)
    res = run_bass_kernel_spmd(nc, in_maps, core_ids=list(range(dims.NC)))
    out = np.concatenate([r["out2"] for r in res.results], axis=0)
    return out.astype(np.float32)


# revision 21
# speedup vs baseline: 2.3861x; 2.3861x over previous
"""2-layer GAT (PyG GATConv style) distributed across 8 TRN2 NeuronCores.

Sharding: nodes partitioned into 8 contiguous destination blocks (graph
parallel); weights replicated.

Per core:
  A. One matmul per 128-node tile against the extended weight matrix
     [W1 | vs1 | vd1] produces xp1 AND both attention logits (als1/ald1)
     as extra columns. Rows [xp1|als1] (260 bf16) go to a 768B-pitch
     gather table; ald1 stays SBUF-resident.
  G. AllGather of the layer-1 table.
  C. Layer-1 edge phase over the core's own destination block, one
     128-dst group at a time. Edges are packed DENSELY (one slot per
     edge, column-major [128 x CO]) sorted by (table-half, src); source
     rows arrive via batched dma_gather (int16 indices, two <32768-row
     table halves). Per-slot dst one-hots st[p,c,d] (DVE iota-compare)
     drive both the segment-sum matmuls on the TensorEngine (PSUM
     accumulation; the softmax normalizer rides as 4 extra rhs columns)
     and - transposed via PE - the per-edge broadcast of ald. The
     epilogue normalizes, applies bias+ELU, and computes the layer-2
     node row [xp2|als2] + SBUF-resident ald2 via [W2 | vs2 | vd2].
  H. AllGather of the (256B-pitch) layer-2 table.
  F. Layer-2 edge phase, same structure (17-wide rows) + log_softmax.

Host preprocessing is index-only (self-loops, dst-block bucketing,
dense slot packing, padding, int16 index images); the only host float
math is on the WEIGHTS (column permutation and folding the attention
vectors into the extended weight matrices); all data-dependent float
math runs on device.
"""

import math
from contextlib import ExitStack

import numpy as np
import ml_dtypes

import concourse.bass as bass
import concourse.tile as tile
from concourse import bacc, mybir
from concourse.bass_utils import run_bass_kernel_spmd
from concourse.masks import make_identity

F32 = mybir.dt.float32
BF16 = mybir.dt.bfloat16
I16 = mybir.dt.int16
AF = mybir.ActivationFunctionType
OP = mybir.AluOpType

P = 128
NEG_SLOPE = 0.2


def dma_gather_raw(gp, out_ap, in_ap, idxs_ap, num_idxs, elem_size,
                   elem_step, single_packet=None):
    """BassGpSimd.dma_gather minus the payload%256 assert (pitch must still
    be a 256B multiple; verified on HW that arbitrary payload works)."""
    from concourse._compat import exact_div
    if single_packet is None:
        single_packet = num_idxs <= 1024
    assert idxs_ap.dtype == mybir.dt.int16
    assert in_ap.dtype == out_ap.dtype
    stride_bytes = elem_step * mybir.dt.size(in_ap.dtype)
    stride_bytes_256 = exact_div(stride_bytes, 256)
    assert stride_bytes_256 < 256
    _in_ap = gp.lower_ap_dma(in_ap, for_custom_bir_dma=True)
    _idxs_ap = gp.lower_ap(idxs_ap)
    _out_ap = gp.lower_ap(out_ap)
    return gp.add_instruction(
        mybir.InstDMAGatherAnt(
            name=gp.bass.get_next_instruction_name(),
            ins=[*_in_ap, _idxs_ap, gp.lower_val_access(gp.to_reg(num_idxs))],
            outs=[_out_ap],
            transpose=False,
            num_idxs=num_idxs,
            elem_size=elem_size,
            stride_bytes_256=stride_bytes_256,
            gen_mode=0,
            single_packet=single_packet,
            queue_num=0,
            sbuf_tokens_per_rank=0,
            sbuf_free_dim_per_rank=0,
            sbuf_free_dim_pad_per_rank=0,
            sbuf_byte_offset=0,
        ))


class Dims:
    def __init__(self, N, E, n_cores, H1=4, C1=64, H2=1, C2=16, F_in=256):
        self.N, self.E, self.NC = N, E, n_cores
        self.F_in = F_in
        self.H1, self.C1, self.H2, self.C2 = H1, C1, H2, C2
        self.D1 = H1 * C1
        self.D2 = H2 * C2
        self.B = N // n_cores
        self.G = math.ceil(self.B / P)
        self.HALF = min(25000, (N + 1) // 2)  # int16 table split point
        self.R1 = self.D1 + self.H1          # gathered row 1: xp1|als1
        self.T1 = 384                         # table-1 pitch (768B bf16)
        self.R2 = self.D2 + self.H2          # gathered row 2: xp2|als2
        self.T2 = 128                         # table-2 pitch (256B bf16)
        self.cA = None   # per-group half-A column counts (common over cores)
        self.cB = None
        self.NI2 = None  # per-group layer-2 ap_gather num_idxs (per band)
        self.BW2 = 784   # layer-2 slab band pitch per chunk (>= ceil(B/8))
        self.M2 = 8 * self.BW2   # layer-2 slab num_elems


def _wrap_idx16(flat):
    """index list -> [128, ceil(n/16)] int16 SBUF image (16-partition wrap,
    replicated for the 8 Q7 cores)."""
    n = len(flat)
    S = math.ceil(n / 16)
    a = np.zeros((16, S), np.int16)
    i = np.arange(n)
    a[i % 16, i // 16] = flat
    return np.tile(a, (8, 1))


def host_prep(dims: Dims, edge_index: np.ndarray):
    """Index-only preprocessing: self-loops, per-core dst blocks, 128-dst
    groups, dense column-major slot packing sorted by (half, src)."""
    N, NC, B, G = dims.N, dims.NC, dims.B, dims.G
    HALF = dims.HALF
    loops = np.arange(N, dtype=np.int64)
    src = np.concatenate([edge_index[0].astype(np.int64), loops])
    dst = np.concatenate([edge_index[1].astype(np.int64), loops])

    # per (core, group): src lists split by table half, sorted by src
    lists = []           # lists[k][g] = (srcA, srcB, dloc_A, dloc_B)
    for k in range(NC):
        lo, hi = k * B, (k + 1) * B
        m = (dst >= lo) & (dst < hi)
        s_k = src[m]
        d_k = dst[m] - lo
        per_g = []
        for g in range(G):
            gm = (d_k // P) == g
            sg, dg = s_k[gm], d_k[gm] - g * P
            ha = sg < HALF
            oa = np.argsort(sg[ha], kind="stable")
            ob = np.argsort(sg[~ha], kind="stable")
            per_g.append((sg[ha][oa], sg[~ha][ob] - HALF,
                          dg[ha][oa], dg[~ha][ob]))
        lists.append(per_g)

    cA = [max(max(1, math.ceil(len(lists[k][g][0]) / P)) for k in range(NC))
          for g in range(G)]
    cB = [max(max(1, math.ceil(len(lists[k][g][1]) / P)) for k in range(NC))
          for g in range(G)]
    dims.cA, dims.cB = cA, cB

    # layer-2 banded slot structure: edge src s -> band (s%B)%8,
    # slab element m = BW2*(s//B) + (s%B)//8
    BW2 = dims.BW2
    bands = []      # bands[k][g][b] = (m_list, dloc_list)
    for k in range(NC):
        per_g = []
        for g in range(G):
            sa, sb, da, db = lists[k][g]
            s_all = np.concatenate([sa, sb + HALF])
            d_all = np.concatenate([da, db])
            bb = (s_all % B) % 8
            mm = BW2 * (s_all // B) + (s_all % B) // 8
            per_b = []
            for b in range(8):
                sel = bb == b
                o = np.argsort(mm[sel], kind="stable")
                per_b.append((mm[sel][o], d_all[sel][o]))
            per_g.append(per_b)
        bands.append(per_g)
    NI2 = []
    for g in range(G):
        n = max(len(bands[k][g][b][0]) for k in range(NC) for b in range(8))
        NI2.append(((n + 3) // 4) * 4)
    dims.NI2 = NI2
    T2g = [math.ceil(n / P) for n in NI2]
    S2 = [2 * math.ceil(n / 32) for n in NI2]
    C2 = [8 * t for t in T2g]

    SA = [c * 8 for c in cA]           # idx image cols per group (n/16)
    SB = [c * 8 for c in cB]
    CO = [a + b for a, b in zip(cA, cB)]
    per_core = []
    for k in range(NC):
        idxA = np.zeros((P, sum(SA)), np.int16)
        idxB = np.zeros((P, sum(SB)), np.int16)
        dstl = np.full((P, sum(CO)), -1.0, np.float32)
        emask = np.full((P, sum(CO)), -150.0, np.float32)
        idx2 = np.full((P, sum(S2)), -1, np.int16)
        dstl2 = np.full((P, sum(C2)), -1.0, np.float32)
        emask2 = np.full((P, sum(C2)), -150.0, np.float32)
        oa = ob = oc = o2 = oc2 = 0
        for g in range(G):
            sa, sb, da, db = lists[k][g]
            nA, nB = cA[g] * P, cB[g] * P
            fa = np.zeros(nA, np.int64)
            fa[:len(sa)] = sa
            fb = np.zeros(nB, np.int64)
            fb[:len(sb)] = sb
            idxA[:, oa:oa + SA[g]] = _wrap_idx16(fa)
            idxB[:, ob:ob + SB[g]] = _wrap_idx16(fb)
            for off, cols, dl, ne in ((0, cA[g], da, len(sa)),
                                      (cA[g], cB[g], db, len(sb))):
                i = np.arange(ne)
                dstl[i % P, oc + off + i // P] = dl
                emask[i % P, oc + off + i // P] = 0.0
            # layer-2 banded images
            ni, t2 = NI2[g], T2g[g]
            for b in range(8):
                ml, dl2 = bands[k][g][b]
                ne = len(ml)
                fm = np.full(ni, -1, np.int64)
                fm[:ne] = ml
                i = np.arange(ni)
                idx2[16 * b + i % 16, o2 + i // 16] = fm
                j = np.arange(ne)
                dstl2[j % P, oc2 + b * t2 + j // P] = dl2
                emask2[j % P, oc2 + b * t2 + j // P] = 0.0
            oa += SA[g]
            ob += SB[g]
            oc += CO[g]
            o2 += S2[g]
            oc2 += C2[g]
        per_core.append(dict(
            idxA=idxA, idxB=idxB,
            dstl=dstl.astype(ml_dtypes.bfloat16),
            emask=emask.astype(ml_dtypes.bfloat16),
            idx2=idx2,
            dstl2=dstl2.astype(ml_dtypes.bfloat16),
            emask2=emask2.astype(ml_dtypes.bfloat16),
        ))
    return per_core


def build_program(dims: Dims):
    N, NC, B, G = dims.N, dims.NC, dims.B, dims.G
    F_in, D1, D2, H1, H2 = dims.F_in, dims.D1, dims.D2, dims.H1, dims.H2
    C1 = dims.C1
    R1, T1, R2, T2 = dims.R1, dims.T1, dims.R2, dims.T2
    cA, cB = dims.cA, dims.cB
    CO = [a + b for a, b in zip(cA, cB)]
    COmax = max(CO)
    SA = [c * 8 for c in cA]
    SB = [c * 8 for c in cB]
    KF = F_in // P
    KD = D1 // P
    HALF = dims.HALF
    E1 = D1 + 2 * H1     # stage-A matmul width (xp1|als1|ald1)
    E2 = D2 + 2 * H2     # layer-2 matmul width (xp2|als2|ald2)

    NI2 = dims.NI2
    T2g = [math.ceil(n / P) for n in NI2]
    S2 = [2 * math.ceil(n / 32) for n in NI2]
    C2 = [8 * t for t in T2g]
    C2max = max(C2)
    NImax = max(NI2)
    BW2, M2 = dims.BW2, dims.M2

    nc = bacc.Bacc("TRN2", target_bir_lowering=False, debug=False,
                   enable_asserts=False, num_devices=NC)

    xT = nc.dram_tensor("xT", [F_in, B], F32, kind="ExternalInput")
    W1e = nc.dram_tensor("W1e", [F_in, E1], F32, kind="ExternalInput")
    b1 = nc.dram_tensor("b1", [D1], F32, kind="ExternalInput")
    W2e = nc.dram_tensor("W2e", [D1, E2], F32, kind="ExternalInput")
    b2 = nc.dram_tensor("b2", [D2], F32, kind="ExternalInput")
    idxA = nc.dram_tensor("idxA", [P, sum(SA)], I16, kind="ExternalInput")
    idxB = nc.dram_tensor("idxB", [P, sum(SB)], I16, kind="ExternalInput")
    dstl = nc.dram_tensor("dstl", [P, sum(CO)], BF16, kind="ExternalInput")
    emask = nc.dram_tensor("emask", [P, sum(CO)], BF16, kind="ExternalInput")
    idx2 = nc.dram_tensor("idx2", [P, sum(S2)], I16, kind="ExternalInput")
    dstl2 = nc.dram_tensor("dstl2", [P, sum(C2)], BF16, kind="ExternalInput")
    emask2 = nc.dram_tensor("emask2", [P, sum(C2)], BF16,
                            kind="ExternalInput")
    out2 = nc.dram_tensor("out2", [B, D2], F32, kind="ExternalOutput")

    t1_loc = nc.dram_tensor("t1_loc", [B, T1], BF16)
    t1_full = nc.dram_tensor("t1_full", [N, T1], BF16, addr_space="Shared")
    TS_R = 32            # f-major staging rows (18 used: xp2|als2)
    t2S_loc = nc.dram_tensor("t2S_loc", [TS_R, M2], BF16)
    t2S_full = nc.dram_tensor("t2S_full", [NC * TS_R, M2], BF16,
                              addr_space="Shared")

    rg = [list(range(NC))]

    with tile.TileContext(nc) as tc, ExitStack() as ctx:
        const = ctx.enter_context(tc.tile_pool(name="const", bufs=1))
        ictx = ExitStack()
        cpsum = ictx.enter_context(tc.tile_pool(name="cpsum", bufs=1,
                                                space="PSUM"))

        iota_i = const.tile([P, P], mybir.dt.int32, tag="iota_i")
        nc.gpsimd.iota(iota_i[:], pattern=[[1, P]], base=0,
                       channel_multiplier=0)
        iota_bf = const.tile([P, P], BF16, tag="iota_bf")
        nc.vector.tensor_copy(iota_bf[:], iota_i[:])
        ident = const.tile([P, P], BF16, tag="ident")
        make_identity(nc, ident[:])

        w1sb = const.tile([P, KF, E1], BF16, tag="w1sb")
        for c in range(KF):
            nc.gpsimd.dma_start(out=w1sb[:, c, :], in_=W1e[c * P:(c + 1) * P, :])
        w2sb = const.tile([P, KD, E2], BF16, tag="w2sb")
        for c in range(KD):
            nc.gpsimd.dma_start(out=w2sb[:, c, :], in_=W2e[c * P:(c + 1) * P, :])

        ones_row = const.tile([1, P], F32, tag="ones_row")
        nc.vector.memset(ones_row[:], 1.0)

        def replicate(vec_ap, X, tag):
            vrow = const.tile([1, X], F32, tag=tag + "_row")
            nc.sync.dma_start(out=vrow[:], in_=vec_ap[None, :])
            pr = cpsum.tile([P, X], F32, tag="reppsum")
            nc.tensor.matmul(out=pr[:], lhsT=ones_row[:], rhs=vrow[:],
                             start=True, stop=True)
            rep = const.tile([P, X], F32, tag=tag)
            nc.vector.tensor_copy(rep[:], pr[:])
            return rep

        b1_r = replicate(b1, D1, "b1_r")
        b2_r = replicate(b2, D2, "b2_r")

        idxA_sb = const.tile([P, sum(SA)], I16, tag="idxA_sb")
        nc.sync.dma_start(out=idxA_sb[:], in_=idxA[:, :])
        idxB_sb = const.tile([P, sum(SB)], I16, tag="idxB_sb")
        nc.sync.dma_start(out=idxB_sb[:], in_=idxB[:, :])
        dstl_sb = const.tile([P, sum(CO)], BF16, tag="dstl_sb")
        nc.sync.dma_start(out=dstl_sb[:], in_=dstl[:, :])
        emask_sb = const.tile([P, sum(CO)], BF16, tag="emask_sb")
        nc.sync.dma_start(out=emask_sb[:], in_=emask[:, :])
        idx2_sb = const.tile([P, sum(S2)], I16, tag="idx2_sb")
        nc.sync.dma_start(out=idx2_sb[:], in_=idx2[:, :])
        dstl2_sb = const.tile([P, sum(C2)], BF16, tag="dstl2_sb")
        nc.sync.dma_start(out=dstl2_sb[:], in_=dstl2[:, :])
        emask2_sb = const.tile([P, sum(C2)], BF16, tag="emask2_sb")
        nc.sync.dma_start(out=emask2_sb[:], in_=emask2[:, :])
        # layer-2 f-major staging (written per-group in C, dumped for AG)
        tS = const.tile([TS_R, M2], BF16, tag="tS")

        # SBUF-resident dst-side logits (written in A/C, read in C/F)
        aldt = const.tile([P, G, H1], BF16, tag="aldt")
        nc.vector.memset(aldt[:], 0.0)
        aldt2 = const.tile([P, G, H2], BF16, tag="aldt2")
        nc.vector.memset(aldt2[:], 0.0)

        ictx.close()

        # ---- stage A: layer-1 node table for own block -----------------
        actx = ExitStack()
        pa = actx.enter_context(tc.tile_pool(name="pa", bufs=3))
        pa_ps = actx.enter_context(tc.tile_pool(name="pa_ps", bufs=2,
                                                space="PSUM"))
        for t in range(G):
            n0 = t * P
            nn = min(P, B - n0)
            xta = pa.tile([P, KF, P], BF16, tag="xta")
            for c in range(KF):
                nc.gpsimd.dma_start(out=xta[:, c, :nn],
                                    in_=xT[c * P:(c + 1) * P, n0:n0 + nn])
            ps_xp = pa_ps.tile([P, E1], F32, tag="ps_xp")
            for c in range(KF):
                nc.tensor.matmul(out=ps_xp[:nn, :], lhsT=xta[:, c, :nn],
                                 rhs=w1sb[:, c, :],
                                 start=(c == 0), stop=(c == KF - 1))
            row = pa.tile([P, R1], BF16, tag="row1")
            nc.vector.tensor_copy(row[:nn, :], ps_xp[:nn, 0:R1])
            nc.vector.tensor_copy(aldt[:nn, t, :], ps_xp[:nn, R1:R1 + H1])
            nc.sync.dma_start(out=t1_loc[n0:n0 + nn, 0:R1], in_=row[:nn, :])
        actx.close()

        # ---- AllGather layer-1 table -----------------------------------
        nc.gpsimd.collective_compute(
            "AllGather", OP.bypass, replica_groups=rg,
            ins=[t1_loc.ap()], outs=[t1_full.ap()])

        # ---- stage C: layer-1 edge phase + fused layer-2 table ---------
        cctx = ExitStack()
        pg = cctx.enter_context(tc.tile_pool(name="pg", bufs=2))
        pm = cctx.enter_context(tc.tile_pool(name="pm", bufs=2))
        pe = cctx.enter_context(tc.tile_pool(name="pe", bufs=2))
        pst = cctx.enter_context(tc.tile_pool(name="pst", bufs=2))
        pt_ps = cctx.enter_context(tc.tile_pool(name="pt_ps", bufs=1,
                                                space="PSUM"))
        pc_ps = cctx.enter_context(tc.tile_pool(name="pc_ps", bufs=1,
                                                space="PSUM"))
        px_ps = cctx.enter_context(tc.tile_pool(name="px_ps", bufs=1,
                                                space="PSUM"))
        oa = ob = oc = 0
        for g in range(G):
            w0 = g * P
            wn = min(P, B - w0)
            ca, cb, co = cA[g], cB[g], CO[g]

            gat = pg.tile([P, COmax, R1], BF16, tag="gat")
            dma_gather_raw(nc.gpsimd, gat[:, 0:ca, :], t1_full[0:HALF, 0:R1],
                           idxA_sb[:, oa:oa + SA[g]], ca * P, R1, T1,
                           single_packet=False)
            dma_gather_raw(nc.gpsimd, gat[:, ca:co, :], t1_full[HALF:N, 0:R1],
                           idxB_sb[:, ob:ob + SB[g]], cb * P, R1, T1,
                           single_packet=False)

            # per-slot dst one-hots (st) and their transposes (stT)
            st = pst.tile([P, COmax, P], BF16, tag="st")
            nc.vector.tensor_tensor(
                out=st[:, 0:co, :],
                in0=iota_bf[:, None, :].to_broadcast([P, co, P]),
                in1=dstl_sb[:, oc:oc + co][:, :, None].to_broadcast(
                    [P, co, P]),
                op=OP.is_equal)
            stT_ps = pt_ps.tile([P, COmax, P], BF16, tag="stT_ps")
            for c in range(co):
                nc.tensor.transpose(stT_ps[:, c, :], st[:, c, :], ident[:])
            stT = pst.tile([P, COmax, P], BF16, tag="stT")
            nc.vector.tensor_copy(stT[:, 0:co, :], stT_ps[:, 0:co, :])

            # ald broadcast to slots: aldE[p,c,:] = ald_g[dst(p,c),:]
            aldE_ps = pt_ps.tile([P, COmax, H1], F32, tag="aldE_ps")
            for c in range(co):
                nc.tensor.matmul(out=aldE_ps[:, c, :], lhsT=stT[:, c, :],
                                 rhs=aldt[:, g, :], start=True, stop=True)
            aldE = pe.tile([P, COmax, H1], BF16, tag="aldE")
            nc.vector.tensor_copy(aldE[:, 0:co, :], aldE_ps[:, 0:co, :])

            # ex = exp(leaky_relu(als[s] + ald[d]) + pad_mask)
            ep = pe.tile([P, COmax, H1], F32, tag="ep")
            nc.vector.tensor_tensor(out=ep[:, 0:co, :],
                                    in0=gat[:, 0:co, D1:D1 + H1],
                                    in1=aldE[:, 0:co, :], op=OP.add)
            lr = pe.tile([P, COmax, H1], F32, tag="lr")
            nc.vector.tensor_scalar_mul(lr[:, 0:co, :], ep[:, 0:co, :],
                                        NEG_SLOPE)
            nc.vector.tensor_tensor(out=lr[:, 0:co, :], in0=lr[:, 0:co, :],
                                    in1=ep[:, 0:co, :], op=OP.max)
            nc.vector.tensor_tensor(
                out=lr[:, 0:co, :], in0=lr[:, 0:co, :],
                in1=emask_sb[:, oc:oc + co][:, :, None].to_broadcast(
                    [P, co, H1]),
                op=OP.add)
            msg = pm.tile([P, COmax, R1], BF16, tag="msg")
            nc.scalar.activation(msg[:, 0:co, D1:D1 + H1], lr[:, 0:co, :],
                                 AF.Exp)
            nc.vector.tensor_tensor(
                out=msg[:, 0:co, 0:D1].rearrange("p k (c h) -> p k c h",
                                                 h=H1),
                in0=gat[:, 0:co, 0:D1].rearrange("p k (c h) -> p k c h",
                                                 h=H1),
                in1=msg[:, 0:co, D1:D1 + H1][:, :, None, :].to_broadcast(
                    [P, co, C1, H1]),
                op=OP.mult)

            ps_g = pc_ps.tile([P, R1], F32, tag="ps_g")
            for c in range(co):
                nc.tensor.matmul(out=ps_g[:], lhsT=st[:, c, :],
                                 rhs=msg[:, c, :],
                                 start=(c == 0), stop=(c == co - 1))

            # epilogue: alpha-normalize, +b1, ELU -> h1 (bf16)
            rec = pe.tile([P, H1], F32, tag="rec")
            nc.vector.reciprocal(rec[:wn], ps_g[:wn, D1:D1 + H1])
            h1f = pg.tile([P, D1], F32, tag="h1f")
            nc.vector.tensor_tensor(
                out=h1f[:wn].rearrange("p (c h) -> p c h", h=H1),
                in0=ps_g[:wn, 0:D1].rearrange("p (c h) -> p c h", h=H1),
                in1=rec[:wn][:, None, :].to_broadcast([wn, C1, H1]),
                op=OP.mult)
            nc.vector.tensor_tensor(out=h1f[:wn], in0=h1f[:wn], in1=b1_r[:wn],
                                    op=OP.add)
            mn = pe.tile([P, D1], F32, tag="mn")
            nc.vector.tensor_scalar_min(mn[:wn], h1f[:wn], 0.0)
            em = pe.tile([P, D1], F32, tag="em")
            nc.scalar.activation(em[:wn], mn[:wn], AF.Exp)
            nc.vector.tensor_tensor(out=h1f[:wn], in0=h1f[:wn], in1=mn[:wn],
                                    op=OP.subtract)
            nc.vector.tensor_scalar_add(em[:wn], em[:wn], -1.0)
            h1b = pg.tile([P, D1], BF16, tag="h1b")
            nc.vector.tensor_tensor(out=h1b[:wn], in0=h1f[:wn], in1=em[:wn],
                                    op=OP.add)

            # fused layer-2 node-table build (xp2|als2|ald2)
            ps_x2 = px_ps.tile([P, E2], F32, tag="ps_x2")
            for c in range(KD):
                pt = px_ps.tile([P, P], BF16, tag="pt")
                nc.tensor.transpose(pt[:], h1b[:, c * P:(c + 1) * P], ident[:])
                cpt = pe.tile([P, P], BF16, tag="cpt")
                nc.vector.tensor_copy(cpt[:], pt[:])
                nc.tensor.matmul(out=ps_x2[:], lhsT=cpt[:], rhs=w2sb[:, c, :],
                                 start=(c == 0), stop=(c == KD - 1))
            nc.vector.tensor_copy(aldt2[:wn, g, :], ps_x2[:wn, R2:R2 + H2])
            x2sb = pe.tile([P, E2], BF16, tag="x2sb")
            nc.vector.tensor_copy(x2sb[:wn, :], ps_x2[:wn, :])
            T2p = px_ps.tile([E2, P], BF16, tag="T2p")
            nc.tensor.transpose(T2p[:], x2sb[:, :], ident[:])
            # band-reorder into the f-major staging: node j of this group
            # goes to column BW2*(j%8) + 16*g + j//8
            nc.vector.tensor_copy(
                tS[0:R2 + H2, :].rearrange(
                    "f (b m) -> f b m", b=8)[:, :, 16 * g:16 * g + 16],
                T2p[0:R2 + H2, :].rearrange("f (m b) -> f b m", b=8))

            oa += SA[g]
            ob += SB[g]
            oc += CO[g]
        cctx.close()

        # ---- AllGather layer-2 f-major staging -------------------------
        nc.sync.dma_start(out=t2S_loc[:, :], in_=tS[:])
        nc.gpsimd.collective_compute(
            "AllGather", OP.bypass, replica_groups=rg,
            ins=[t2S_loc.ap()], outs=[t2S_full.ap()])

        # ---- stage F: layer-2 edge phase + log_softmax ------------------
        fctx = ExitStack()
        pslab = fctx.enter_context(tc.tile_pool(name="pslab", bufs=1))
        pf = fctx.enter_context(tc.tile_pool(name="pf", bufs=2))
        pfs = fctx.enter_context(tc.tile_pool(name="pfs", bufs=2))
        pf2_ps = fctx.enter_context(tc.tile_pool(name="pf2_ps", bufs=1,
                                                 space="PSUM"))
        pf_ps = fctx.enter_context(tc.tile_pool(name="pf_ps", bufs=2,
                                                space="PSUM"))

        # stitch the AllGathered staging into the banded f-major slab:
        # slab[16b+r, BW2*k + m, dp] = t2S_full[TS_R*k + r + 16*dp, BW2*b + m]
        slab2 = pslab.tile([P, M2, 2], BF16, tag="slab2")
        sctx = ExitStack()
        pstg = sctx.enter_context(tc.tile_pool(name="pstg", bufs=1))
        stg = pstg.tile([P, 2, M2], BF16, tag="stg")
        nc.vector.memset(stg[:, 1, :], 0.0)
        for b in range(8):
            for dp in range(2):
                rr = 16 if dp == 0 else (R2 + H2 - 16)
                for k in range(NC):
                    nc.sync.dma_start(
                        out=stg[16 * b:16 * b + rr, dp,
                                BW2 * k:BW2 * (k + 1)],
                        in_=t2S_full[TS_R * k + 16 * dp:
                                     TS_R * k + 16 * dp + rr,
                                     BW2 * b:BW2 * (b + 1)])
        nc.vector.tensor_copy(slab2[:, :, 0], stg[:, 0, :])
        nc.vector.tensor_copy(slab2[:, :, 1], stg[:, 1, :])
        sctx.close()

        o2 = oc2 = 0
        for g in range(G):
            w0 = g * P
            wn = min(P, B - w0)
            ni, t2c, c2 = NI2[g], T2g[g], C2[g]

            g2 = pf.tile([P, max(T2g) * P, 2], BF16, tag="g2")
            if ni < t2c * P:
                nc.vector.memset(g2[:, ni:t2c * P, :], 0.0)
            nc.gpsimd.ap_gather(g2[:, 0:ni, :], slab2[:],
                                idx2_sb[:, o2:o2 + S2[g]], channels=P,
                                num_elems=M2, d=2, num_idxs=ni)

            # transpose banded f-major -> slot-major: one full [128,128]
            # transpose per (tile, dpos); out[j, 16b+r] = band-b value r
            psT = pf2_ps.tile([P, max(T2g), 2, P], BF16, tag="psT")
            for t in range(t2c):
                for dp in range(2):
                    nc.tensor.transpose(psT[:, t, dp, :],
                                        g2[:, t * P:(t + 1) * P, dp],
                                        ident[:])
            gat2 = pf.tile([P, C2max, R2], BF16, tag="gat2")
            # gat2[:, b*t2c+t, r] = psT[:, t, 0, 16b+r]  (features 0..15)
            nc.vector.tensor_copy(
                gat2[:, 0:c2, 0:16].rearrange("p (b t) f -> p b t f", b=8),
                psT[:, 0:t2c, 0, :].rearrange("p t (b f) -> p b t f", b=8))
            # feature 16 comes from dpos=1, r=0
            nc.vector.tensor_copy(
                gat2[:, 0:c2, 16:17].rearrange("p (b t) f -> p b t f", b=8),
                psT[:, 0:t2c, 1, :].rearrange(
                    "p t (b f) -> p b t f", b=8)[:, :, :, 0:1])

            st2 = pfs.tile([P, C2max, P], BF16, tag="st2")
            nc.vector.tensor_tensor(
                out=st2[:, 0:c2, :],
                in0=iota_bf[:, None, :].to_broadcast([P, c2, P]),
                in1=dstl2_sb[:, oc2:oc2 + c2][:, :, None].to_broadcast(
                    [P, c2, P]),
                op=OP.is_equal)
            stT2_ps = pf2_ps.tile([P, C2max, P], BF16, tag="stT2_ps")
            for c in range(c2):
                nc.tensor.transpose(stT2_ps[:, c, :], st2[:, c, :], ident[:])
            stT2 = pfs.tile([P, C2max, P], BF16, tag="stT2")
            nc.vector.tensor_copy(stT2[:, 0:c2, :], stT2_ps[:, 0:c2, :])

            ald2E_ps = pf2_ps.tile([P, C2max, H2], F32, tag="ald2E_ps")
            for c in range(c2):
                nc.tensor.matmul(out=ald2E_ps[:, c, :], lhsT=stT2[:, c, :],
                                 rhs=aldt2[:, g, :], start=True, stop=True)
            ald2E = pf.tile([P, C2max, H2], BF16, tag="ald2E")
            nc.vector.tensor_copy(ald2E[:, 0:c2, :], ald2E_ps[:, 0:c2, :])

            ep2 = pf.tile([P, C2max, H2], F32, tag="ep2")
            nc.vector.tensor_tensor(out=ep2[:, 0:c2, :],
                                    in0=gat2[:, 0:c2, D2:D2 + H2],
                                    in1=ald2E[:, 0:c2, :], op=OP.add)
            lr2 = pf.tile([P, C2max, H2], F32, tag="lr2")
            nc.vector.tensor_scalar_mul(lr2[:, 0:c2, :], ep2[:, 0:c2, :],
                                        NEG_SLOPE)
            nc.vector.tensor_tensor(out=lr2[:, 0:c2, :], in0=lr2[:, 0:c2, :],
                                    in1=ep2[:, 0:c2, :], op=OP.max)
            nc.vector.tensor_tensor(
                out=lr2[:, 0:c2, :], in0=lr2[:, 0:c2, :],
                in1=emask2_sb[:, oc2:oc2 + c2][:, :, None].to_broadcast(
                    [P, c2, H2]),
                op=OP.add)
            msg2 = pf.tile([P, C2max, R2], BF16, tag="msg2")
            nc.scalar.activation(msg2[:, 0:c2, D2:D2 + H2], lr2[:, 0:c2, :],
                                 AF.Exp)
            nc.vector.tensor_tensor(
                out=msg2[:, 0:c2, 0:D2],
                in0=gat2[:, 0:c2, 0:D2],
                in1=msg2[:, 0:c2, D2:D2 + H2].to_broadcast([P, c2, D2]),
                op=OP.mult)

            ps2 = pf_ps.tile([P, R2], F32, tag="ps2")
            for c in range(c2):
                nc.tensor.matmul(out=ps2[:], lhsT=st2[:, c, :],
                                 rhs=msg2[:, c, :],
                                 start=(c == 0), stop=(c == c2 - 1))

            rec2 = pf.tile([P, H2], F32, tag="rec2")
            nc.vector.reciprocal(rec2[:wn], ps2[:wn, D2:D2 + H2])
            x2 = pf.tile([P, D2], F32, tag="x2")
            nc.vector.tensor_scalar(out=x2[:wn], in0=ps2[:wn, 0:D2],
                                    scalar1=rec2[:wn], scalar2=None,
                                    op0=OP.mult)
            nc.vector.tensor_tensor(out=x2[:wn], in0=x2[:wn], in1=b2_r[:wn],
                                    op=OP.add)
            mx = pf.tile([P, 1], F32, tag="mx")
            nc.vector.tensor_reduce(out=mx[:wn], in_=x2[:wn],
                                    axis=mybir.AxisListType.X, op=OP.max)
            xs = pf.tile([P, D2], F32, tag="xs")
            nc.vector.tensor_scalar(out=xs[:wn], in0=x2[:wn], scalar1=mx[:wn],
                                    scalar2=None, op0=OP.subtract)
            es = pf.tile([P, D2], F32, tag="es")
            ssum = pf.tile([P, 1], F32, tag="ssum")
            nc.scalar.activation(es[:wn], xs[:wn], AF.Exp, accum_out=ssum[:wn])
            ls = pf.tile([P, 1], F32, tag="ls")
            nc.scalar.activation(ls[:wn], ssum[:wn], AF.Ln)
            ot = pf.tile([P, D2], F32, tag="ot")
            nc.vector.tensor_scalar(out=ot[:wn], in0=xs[:wn], scalar1=ls[:wn],
                                    scalar2=None, op0=OP.subtract)
            nc.sync.dma_start(out=out2[w0:w0 + wn, :], in_=ot[:wn, :])

            o2 += S2[g]
            oc2 += C2[g]
        fctx.close()

    nc.compile()
    return nc


def make_in_maps(dims: Dims, inputs: dict, per_core_meta):
    """Per-core input maps. Weight transforms (host, parameters only):
    columns reordered to the (c,h)-interleaved layout and the attention
    vectors folded in as extra columns of the extended weight matrices."""
    H1, C1, D1, D2 = dims.H1, dims.C1, dims.D1, dims.D2
    perm = np.arange(D1).reshape(H1, C1).T.reshape(-1)   # [h*C+c] -> [c*H+h]
    x = np.asarray(inputs["x"], dtype=np.float32)
    W1 = np.asarray(inputs["W1"], np.float32)
    W2 = np.asarray(inputs["W2"], np.float32)
    a1s = np.asarray(inputs["a1_src"], np.float32)       # [H1, C1]
    a1d = np.asarray(inputs["a1_dst"], np.float32)
    a2s = np.asarray(inputs["a2_src"], np.float32).reshape(-1)   # [D2]
    a2d = np.asarray(inputs["a2_dst"], np.float32).reshape(-1)
    # als1[n,h] = sum_c xp1[n, h*C1+c] * a1s[h,c]  ->  vs1 = W1 @ a1s-fold
    vs1 = np.stack([W1[:, h * C1:(h + 1) * C1] @ a1s[h] for h in range(H1)],
                   axis=1)
    vd1 = np.stack([W1[:, h * C1:(h + 1) * C1] @ a1d[h] for h in range(H1)],
                   axis=1)
    W1e = np.concatenate([W1[:, perm], vs1, vd1], axis=1)
    W2p = W2[perm, :]
    W2e = np.concatenate([W2p, (W2p @ a2s)[:, None], (W2p @ a2d)[:, None]],
                         axis=1)
    reps = {
        "W1e": np.ascontiguousarray(W1e),
        "b1": np.ascontiguousarray(
            np.asarray(inputs["b1"], np.float32).reshape(-1)[perm]),
        "W2e": np.ascontiguousarray(W2e),
        "b2": np.asarray(inputs["b2"], np.float32).reshape(-1),
    }
    in_maps = []
    B = dims.B
    for k in range(dims.NC):
        m = dict(reps)
        m["xT"] = np.ascontiguousarray(x[k * B:(k + 1) * B, :].T)
        m.update(per_core_meta[k])
        in_maps.append(m)
    return in_maps


_CACHE = {}


def _get_program(dims: Dims):
    key = (dims.N, dims.E, dims.NC, tuple(dims.cA), tuple(dims.cB))
    if key not in _CACHE:
        _CACHE[key] = build_program(dims)
    return _CACHE[key]


def kernel(x: np.ndarray, edge_index: np.ndarray, W1, a1_src, a1_dst, b1,
           W2, a2_src, a2_dst, b2) -> np.ndarray:
    x = np.asarray(x)
    edge_index = np.asarray(edge_index)
    dims = Dims(N=x.shape[0], E=edge_index.shape[1], n_cores=8)
    per_core = host_prep(dims, edge_index)
    nc = _get_program(dims)
    in_maps = make_in_maps(
        dims,
        dict(x=x, edge_index=edge_index, W1=W1, a1_src=a1_src, a1_dst=a1_dst,
             b1=b1, W2=W2, a2_src=a2_src, a2_dst=a2_dst, b2=b2),
        per_core)
    res = run_bass_kernel_spmd(nc, in_maps, core_ids=list(range(dims.NC)))
    out = np.concatenate([r["out2"] for r in res.results], axis=0)
    return out.astype(np.float32)
